# revision 7
# baseline (speedup 1.0000x reference)
"""AnomalyAttention Trainium2 kernel — 8 NeuronCores, data-parallel over batch.

Computes, for B=32, L=512, H=8, E=64 (shapes hardcoded):
    scores   = Q @ K^T (causal masked), series = softmax(scores/8)
    V_out    = series @ V
    prior    = 1/(sqrt(2pi) sig) * exp(-(i-j)^2 / (2 sig^2))
    sigma_out= broadcast(sig)  with sig = 3^(sigmoid(5*sigma)+1e-5) - 1

Each of the 8 cores handles 4 batches; all heads computed locally, no
collectives. QK^T and series@V run on the TensorEngine in bf16 (fp32
accumulation), exp/prior on the ScalarEngine, normalization on the
VectorEngine. The causal structure skips all fully-masked score blocks.
"""
import math
import sys
import types
from contextlib import ExitStack

sys.path.insert(0, "/opt/trn_rl_repo")

import numpy as np

# NTFF profile hook shim: the container's antenv package lacks axon_hooks, so
# register an equivalent module before concourse imports it (trace=True path).
if "antenv.axon_hooks" not in sys.modules:
    _hook_mod = types.ModuleType("antenv.axon_hooks")
    _hook_store = [None]
    _hook_mod.set_axon_ntff_profile_hook = lambda h: _hook_store.__setitem__(0, h)
    _hook_mod.get_axon_ntff_profile_hook = lambda: _hook_store[0]
    sys.modules["antenv.axon_hooks"] = _hook_mod
    try:
        import antenv

        antenv.axon_hooks = _hook_mod
        from trn_agent_boot.trn_boot import _ntff_profile_via_ctypes

        _hook = _ntff_profile_via_ctypes("/opt/axon/libaxon_pjrt.so")
        if _hook is not None:
            _hook_mod.set_axon_ntff_profile_hook(_hook)
    except Exception:
        pass

import concourse.bass as bass
import concourse.tile as tile
from concourse import mybir
from concourse.bass_utils import run_bass_kernel_spmd
from concourse.masks import make_causal_mask, make_identity
from concourse.vector_clock import ScopedClock

F32 = mybir.dt.float32
BF16 = mybir.dt.bfloat16
ACT = mybir.ActivationFunctionType

N_CORES = 8
B, L, H, E = 32, 512, 8, 64
BL = B // N_CORES  # batches per core
P = 128
NB = L // P  # 4 row blocks
LN3 = math.log(3.0)
NEG_HALF_LN_2PI = -0.5 * math.log(2.0 * math.pi)


def _split_excess_waits(nc):
    """This container's walrus accepts at most one sync-wait per instruction
    (two for EventSemaphore), but Tile attaches one wait per dependency.
    Hoist excess waits onto dedicated same-engine NOPs placed immediately
    before the instruction — equivalent for monotone (sem-ge) waits."""
    fixn = [0]
    for f in nc.m.functions:
        for bb in f.blocks:
            out = []
            changed = False
            for inst in bb.instructions:
                si = inst.sync_info
                n = len(si.on_wait) if si and si.on_wait else 0
                cap = 2 if isinstance(inst, mybir.InstEventSemaphore) else 1
                if n > cap:
                    waits = list(si.on_wait)
                    # keep non-monotone (eq) waits on the instruction itself
                    waits.sort(key=lambda w: "ge" in w.wait_mode)
                    keep, hoist = waits[:cap], waits[cap:]
                    for w in hoist:
                        assert "ge" in w.wait_mode, w
                        fixn[0] += 1
                        nop = mybir.InstNoOp(
                            name=f"Iwfix-{fixn[0]}",
                            engine=inst.engine,
                            ins=[],
                            outs=[],
                            bass_nofuse=True,
                        )
                        nop.sync_info = type(si)(on_wait=[w], on_update=[])
                        out.append(nop)
                    si.on_wait = keep
                    changed = True
                out.append(inst)
            if changed:
                bb.instructions = out
    return nc


def _build():
    nc = bass.Bass("TRN2")
    q = nc.declare_dram_parameter("q", [BL, L, H * E], F32, isOutput=False)
    k = nc.declare_dram_parameter("k", [BL, L, H * E], F32, isOutput=False)
    v = nc.declare_dram_parameter("v", [BL, L, H * E], F32, isOutput=False)
    sg = nc.declare_dram_parameter("sg", [BL, L, H], F32, isOutput=False)
    d2 = nc.declare_dram_parameter("d2", [L, L], F32, isOutput=False)
    vo = nc.declare_dram_parameter("vo", [BL, L, H * E], F32, isOutput=True)
    so = nc.declare_dram_parameter("so", [BL, H, L, L], F32, isOutput=True)
    po = nc.declare_dram_parameter("po", [BL, H, L, L], F32, isOutput=True)
    go = nc.declare_dram_parameter("go", [BL, H, L, L], F32, isOutput=True)

    with ExitStack() as ctx:
        tc = ctx.enter_context(tile.TileContext(nc))
        consts = ctx.enter_context(tc.tile_pool(name="consts", bufs=1))
        slabs = ctx.enter_context(tc.tile_pool(name="slabs", bufs=2))
        ld = ctx.enter_context(tc.tile_pool(name="ld", bufs=3))
        work = ctx.enter_context(tc.tile_pool(name="work", bufs=3))
        eTp = ctx.enter_context(tc.tile_pool(name="eTp", bufs=6))
        small = ctx.enter_context(tc.tile_pool(name="small", bufs=8))
        ps_sc = ctx.enter_context(tc.tile_pool(name="ps_sc", bufs=2, space="PSUM"))
        ps_tr = ctx.enter_context(tc.tile_pool(name="ps_tr", bufs=3, space="PSUM"))
        ps_av = ctx.enter_context(tc.tile_pool(name="ps_av", bufs=2, space="PSUM"))

        cmask = consts.tile([P, P], F32)
        make_causal_mask(nc, cmask, mask_val=-1e30)
        ident = consts.tile([P, P], BF16)
        make_identity(nc, ident)
        zerot = consts.tile([P, L - P], F32)
        nc.vector.memset(zerot, 0.0)
        bias_ln3eps = consts.tile([P, 1], F32)
        nc.vector.memset(bias_ln3eps, 1e-5 * LN3)
        d2t = []
        for i in range(NB):
            t = consts.tile([P, L], F32, tag=f"d2_{i}")
            nc.sync.dma_start(out=t, in_=d2[i * P : (i + 1) * P, :])
            d2t.append(t)

        for b in range(BL):
            # ---- load Q/K/V slabs (all heads), cast to bf16 ----
            qb, kb, vb = [], [], []
            for t in range(NB):
                for src, dst, nm in ((q, qb, "q"), (k, kb, "k"), (v, vb, "v")):
                    f32t = ld.tile([P, L], F32, tag="ld")
                    nc.sync.dma_start(out=f32t, in_=src[b, t * P : (t + 1) * P, :])
                    bft = slabs.tile([P, L], BF16, tag=f"{nm}b{t}")
                    nc.gpsimd.tensor_copy(out=bft, in_=f32t)
                    dst.append(bft)
            # ---- transpose Q/K per head-pair: [L, 2E] -> [2E, L] ----
            qT, kT = [], []
            for hp in range(NB):
                qTt = slabs.tile([P, L], BF16, tag=f"qT{hp}")
                kTt = slabs.tile([P, L], BF16, tag=f"kT{hp}")
                for t in range(NB):
                    for srcl, dstt, nm in ((qb, qTt, "q"), (kb, kTt, "k")):
                        pt = ps_tr.tile([P, P], BF16, tag="ps_tr")
                        nc.tensor.transpose(pt, srcl[t][:, hp * P : (hp + 1) * P], ident)
                        nc.vector.tensor_copy(out=dstt[:, t * P : (t + 1) * P], in_=pt)
                qT.append(qTt)
                kT.append(kTt)
            # ---- sigma chain per L-block: sig, -1/(2 sig^2), ln(c) ----
            sig_t, ns_t, lc_t = [], [], []
            for t in range(NB):
                sraw = small.tile([P, H], F32, tag="sraw")
                nc.sync.dma_start(out=sraw, in_=sg[b, t * P : (t + 1) * P, :])
                s1 = small.tile([P, H], F32, tag="s1")
                nc.scalar.activation(out=s1, in_=sraw, func=ACT.Sigmoid, scale=5.0)
                u = small.tile([P, H], F32, tag="u")
                nc.scalar.activation(
                    out=u, in_=s1, func=ACT.Exp, scale=LN3, bias=bias_ln3eps
                )
                sig = slabs.tile([P, H], F32, tag=f"sig{t}")
                nc.vector.tensor_scalar_add(out=sig, in0=u, scalar1=-1.0)
                sq = small.tile([P, H], F32, tag="sq")
                nc.vector.tensor_mul(out=sq, in0=sig, in1=sig)
                rq = small.tile([P, H], F32, tag="rq")
                nc.vector.reciprocal(out=rq, in_=sq)
                ns = slabs.tile([P, H], F32, tag=f"ns{t}")
                nc.vector.tensor_scalar_mul(out=ns, in0=rq, scalar1=-0.5)
                ln = small.tile([P, H], F32, tag="ln")
                nc.scalar.activation(out=ln, in_=sig, func=ACT.Ln)
                lc = slabs.tile([P, H], F32, tag=f"lc{t}")
                nc.vector.tensor_scalar(
                    out=lc,
                    in0=ln,
                    scalar1=-1.0,
                    scalar2=NEG_HALF_LN_2PI,
                    op0=mybir.AluOpType.mult,
                    op1=mybir.AluOpType.add,
                )
                sig_t.append(sig)
                ns_t.append(ns)
                lc_t.append(lc)

            # ---- per (head, row-block) attention + prior ----
            for h in range(H):
                hp, ho = h // 2, (h % 2) * E
                for i in range(NB):
                    W = (i + 1) * P  # causal width of this row block
                    rows = slice(i * P, (i + 1) * P)
                    sc = ps_sc.tile([P, L], F32, tag="sc")
                    nc.tensor.matmul(
                        sc[:, :W],
                        lhsT=qT[hp][ho : ho + E, rows],
                        rhs=kT[hp][ho : ho + E, :W],
                        start=True,
                        stop=True,
                    )
                    nc.vector.tensor_add(
                        out=sc[:, i * P : W], in0=sc[:, i * P : W], in1=cmask
                    )
                    expb = work.tile([P, L], BF16, tag="exp")
                    rsum = small.tile([P, 1], F32, tag="rsum")
                    nc.scalar.activation(
                        out=expb[:, :W],
                        in_=sc[:, :W],
                        func=ACT.Exp,
                        scale=0.125,
                        accum_out=rsum,
                    )
                    rinv = small.tile([P, 1], F32, tag="rinv")
                    nc.vector.reciprocal(out=rinv, in_=rsum)
                    serf = work.tile([P, L], F32, tag="ser")
                    nc.vector.tensor_scalar_mul(
                        out=serf[:, :W], in0=expb[:, :W], scalar1=rinv
                    )
                    nc.sync.dma_start(out=so[b, h, rows, :W], in_=serf[:, :W])
                    if W < L:
                        nc.sync.dma_start(
                            out=so[b, h, rows, W:], in_=zerot[:, : L - W]
                        )
                    # series @ V via transposed unnormalized exp blocks
                    eTs = []
                    for j in range(i + 1):
                        pt = ps_tr.tile([P, P], BF16, tag="ps_tr")
                        nc.tensor.transpose(
                            pt, expb[:, j * P : (j + 1) * P], ident
                        )
                        eT = eTp.tile([P, P], BF16, tag="eT")
                        nc.vector.tensor_copy(out=eT, in_=pt)
                        eTs.append(eT)
                    va = ps_av.tile([P, E], F32, tag="va")
                    for j in range(i + 1):
                        nc.tensor.matmul(
                            va,
                            lhsT=eTs[j],
                            rhs=vb[j][:, h * E : (h + 1) * E],
                            start=(j == 0),
                            stop=(j == i),
                        )
                    vos = work.tile([P, E], F32, tag="vo")
                    nc.vector.tensor_scalar_mul(out=vos, in0=va, scalar1=rinv)
                    nc.sync.dma_start(
                        out=vo[b, rows, h * E : (h + 1) * E], in_=vos
                    )
                    # prior: exp(d2 * (-1/(2 sig^2)) + ln c) in one ACT pass
                    pri = work.tile([P, L], F32, tag="pri")
                    nc.scalar.activation(
                        out=pri,
                        in_=d2t[i],
                        func=ACT.Exp,
                        scale=ns_t[i][:, h : h + 1],
                        bias=lc_t[i][:, h : h + 1],
                    )
                    nc.sync.dma_start(out=po[b, h, rows, :], in_=pri)
                    # sigma_out: broadcast sig along the row (ACT copy)
                    sgo = work.tile([P, L], F32, tag="sgo")
                    nc.scalar.activation(
                        out=sgo,
                        in_=sig_t[i][:, h : h + 1].to_broadcast([P, L]),
                        func=ACT.Copy,
                    )
                    nc.sync.dma_start(out=go[b, h, rows, :], in_=sgo)
    return _split_excess_waits(nc)


_nc_cache = None
last_results = None


def kernel(queries, keys, values, sigma, attention_mask=None, **_unused):
    """Full-input entry point: shard over 8 cores, run, gather."""
    global _nc_cache, last_results
    if _nc_cache is None:
        _nc_cache = _build()
    nc = _nc_cache

    queries = np.ascontiguousarray(np.asarray(queries), dtype=np.float32)
    keys = np.ascontiguousarray(np.asarray(keys), dtype=np.float32)
    values = np.ascontiguousarray(np.asarray(values), dtype=np.float32)
    sigma = np.ascontiguousarray(np.asarray(sigma), dtype=np.float32)

    idx = np.arange(L, dtype=np.float32)
    d2 = (idx[:, None] - idx[None, :]) ** 2

    in_maps = []
    for c in range(N_CORES):
        bs = slice(c * BL, (c + 1) * BL)
        in_maps.append(
            {
                "q": queries[bs].reshape(BL, L, H * E),
                "k": keys[bs].reshape(BL, L, H * E),
                "v": values[bs].reshape(BL, L, H * E),
                "sg": sigma[bs],
                "d2": d2,
            }
        )

    res = run_bass_kernel_spmd(nc, in_maps, core_ids=list(range(N_CORES)))
    last_results = res

    V = np.concatenate(
        [res.results[c]["vo"].reshape(BL, L, H, E) for c in range(N_CORES)], axis=0
    )
    series = np.concatenate([res.results[c]["so"] for c in range(N_CORES)], axis=0)
    prior = np.concatenate([res.results[c]["po"] for c in range(N_CORES)], axis=0)
    sigma_out = np.concatenate([res.results[c]["go"] for c in range(N_CORES)], axis=0)
    return V, series, prior, sigma_out
